# revision 10
# baseline (speedup 1.0000x reference)
"""BalanceCrossEntropyLoss on 8 Trainium2 NeuronCores.

Problem shapes (hardcoded): pred (16,1,1024,1024) f32, gt (16,1,1024,1024) f32,
mask (16,1024,1024) f32. Output: scalar f32.

Strategy
--------
Data-parallel over the flattened 16M elements: each of the 8 cores gets a
contiguous [128, 16384] f32 shard of pred and gt.

For binary gt and all-ones mask (the spec's fill types), the reference's
hard-negative top-k selects *all* negatives whenever
#neg <= floor(3 * #pos) (true with overwhelming probability for p=0.5
Bernoulli gt), because every negative-pixel loss is > 0 and all other entries
of the flattened negative-loss map are exactly 0.  So:

    negative_sum  = sum(negative_loss)      (no sort/top-k needed)
    balance_loss  = (sum(pos_loss) + negative_sum) / (#pos + #neg + eps)

With g in {0,1}, m == 1:
    pos_loss  = g * (-ln(p+eps) * e^-p)         = -B*g,  B = ln(p+eps)*e^-p
    neg_loss  = (1-g) * (-ln(1-p+eps) * e^(p-1)) = -(A - A*g), A = ln(1-p+eps)*e^(p-1)

Per core the kernel computes four scalars-per-partition streams:
    sum(A), sum(A*g), sum(B*g)   via VectorE tensor_tensor_reduce (fused
                                 product + free-dim reduction)
    sum(g)                       via TensorE ones^T @ g accumulated in PSUM
with A,B built from 4 ScalarE activations (Ln, Ln, Exp, Exp w/ scale+bias).

Host preconditions (mask all-ones, gt binary, #neg <= floor(3*#pos)) are
checked; any violation falls back to an exact numpy implementation of the
reference (including the true top-k).
"""

import sys

sys.path.insert(0, "/opt/trn_rl_repo")

import numpy as np

N_CORES = 8
P = 128
FREE = 16384          # per-core free dim: 16M / 8 cores / 128 partitions
F = 2048              # tile free dim
NT = FREE // F
MM_N = 512            # matmul free-dim chunk (one PSUM bank)
TOTAL = 16 * 1024 * 1024
LOG_EPS = 1e-37
NEGATIVE_RATIO = 3.0
EPS = 1e-6

_NC_CACHE = {}


def _build_nc(free=FREE, tile_f=F, debug=False):
    import concourse.bacc as bacc
    import concourse.mybir as mybir
    from concourse.tile import TileContext

    f32 = mybir.dt.float32
    AF = mybir.ActivationFunctionType
    ALU = mybir.AluOpType
    NT = free // tile_f
    F = tile_f
    MM_N = min(512, F)

    nc = bacc.Bacc(None, target_bir_lowering=False, debug=debug)
    pred = nc.declare_dram_parameter("pred", [P, free], f32, isOutput=False)
    gt = nc.declare_dram_parameter("gt", [P, free], f32, isOutput=False)
    # acc columns: [0:NT]=sum(A), [NT:2NT]=sum(A*g), [2NT:3NT]=sum(B*g),
    # [3NT:4NT]=sum(B) (unused by host, accum_out is mandatory)
    acc_out = nc.declare_dram_parameter("acc", [P, 4 * NT], f32, isOutput=True)
    gsum_out = nc.declare_dram_parameter("gsum", [P, 1], f32, isOutput=True)

    with TileContext(nc) as tc:
        with (
            tc.tile_pool(name="const", bufs=1) as cpool,
            tc.tile_pool(name="io", bufs=3) as io,
            tc.tile_pool(name="work", bufs=2) as work,
            tc.tile_pool(name="psum", bufs=1, space="PSUM") as pp,
        ):
            ones = cpool.tile([P, P], f32)
            nc.vector.memset(ones[:], 1.0)
            acc = cpool.tile([P, 4 * NT], f32)
            ps = pp.tile([P, MM_N], f32)

            def const_ap(val, tag):
                t = cpool.tile([P, 1], f32, tag=tag)
                nc.vector.memset(t[:], val)
                return t

            c_eps = const_ap(LOG_EPS, "c_eps")
            c_neg1 = const_ap(-1.0, "c_neg1")

            for i in range(NT):
                sl = slice(i * F, (i + 1) * F)  # noqa: F841 (slice per tile)
                p_t = io.tile([P, F], f32, tag="p")
                g_t = io.tile([P, F], f32, tag="g")
                nc.sync.dma_start(out=p_t[:], in_=pred[:, sl])
                nc.sync.dma_start(out=g_t[:], in_=gt[:, sl])

                ln1p = work.tile([P, F], f32, tag="ln1p")
                lp = work.tile([P, F], f32, tag="lp")
                ep1 = work.tile([P, F], f32, tag="ep1")
                enp = work.tile([P, F], f32, tag="enp")
                A = work.tile([P, F], f32, tag="A")
                B = work.tile([P, F], f32, tag="B")

                # ln(1-p) (the +1e-37 is absorbed by f32 rounding, matching
                # the reference's own f32 arithmetic), ln(p+1e-37), e^(p-1), e^-p
                nc.scalar.activation(ln1p[:], p_t[:], AF.Ln, bias=1.0, scale=c_neg1[:])
                nc.scalar.activation(lp[:], p_t[:], AF.Ln, bias=c_eps[:], scale=1.0)
                nc.scalar.activation(ep1[:], p_t[:], AF.Exp, bias=c_neg1[:], scale=1.0)
                nc.scalar.activation(enp[:], p_t[:], AF.Exp, bias=0.0, scale=c_neg1[:])

                # A = ln1p*ep1 (+ running sum); A*g; B = lp*enp; B*g
                # (affine_mul_reduce: out = (in0*scale+bias)*in1, accum = sum(out))
                nc.vector.affine_mul_reduce(
                    out=A[:], in0=ln1p[:], in1=ep1[:], scale=1.0, bias=0.0,
                    accum_out=acc[:, 0 * NT + i : 0 * NT + i + 1])
                nc.vector.affine_mul_reduce(
                    out=ep1[:], in0=A[:], in1=g_t[:], scale=1.0, bias=0.0,
                    accum_out=acc[:, 1 * NT + i : 1 * NT + i + 1])
                nc.vector.affine_mul_reduce(
                    out=B[:], in0=lp[:], in1=enp[:], scale=1.0, bias=0.0,
                    accum_out=acc[:, 3 * NT + i : 3 * NT + i + 1])
                nc.vector.affine_mul_reduce(
                    out=enp[:], in0=B[:], in1=g_t[:], scale=1.0, bias=0.0,
                    accum_out=acc[:, 2 * NT + i : 2 * NT + i + 1])

                # sum(g): ones^T @ g chunks accumulated into one PSUM bank
                for c in range(F // MM_N):
                    nc.tensor.matmul(
                        ps[:, :], ones[:, :], g_t[:, c * MM_N : (c + 1) * MM_N],
                        start=(i == 0 and c == 0),
                        stop=(i == NT - 1 and c == F // MM_N - 1))

            # every psum row holds the same per-column counts; reduce free dim
            gs_t = cpool.tile([P, 1], f32)
            nc.vector.reduce_sum(out=gs_t[:], in_=ps[:, :],
                                 axis=mybir.AxisListType.X)
            nc.sync.dma_start(out=acc_out[:, :], in_=acc[:])
            nc.sync.dma_start(out=gsum_out[:, :], in_=gs_t[:])

    nc.finalize()
    return nc


def _get_nc():
    if "nc" not in _NC_CACHE:
        _NC_CACHE["nc"] = _build_nc()
    return _NC_CACHE["nc"]


def _device_sums(pred32, gt32, trace=False, tmpdir=None):
    """pred32/gt32: (8,128,16384) f32. Returns (SA, SAg, SBg, GS, results)."""
    from concourse.bass_utils import run_bass_kernel_spmd

    nc = _get_nc()
    in_maps = [{"pred": pred32[c], "gt": gt32[c]} for c in range(N_CORES)]
    res = run_bass_kernel_spmd(
        nc, in_maps, core_ids=list(range(N_CORES)), trace=trace, tmpdir=tmpdir)
    SA = SAg = SBg = GS = 0.0
    for c in range(N_CORES):
        a = res.results[c]["acc"].astype(np.float64)
        SA += a[:, 0 * NT : 1 * NT].sum()
        SAg += a[:, 1 * NT : 2 * NT].sum()
        SBg += a[:, 2 * NT : 3 * NT].sum()
        GS += float(res.results[c]["gsum"][0, 0])
    return SA, SAg, SBg, GS, res


def _fallback(pred, gt, mask):
    """Exact numpy mirror of the reference (handles arbitrary inputs)."""
    p = pred[:, 0].astype(np.float64)
    g = gt[:, 0].astype(np.float64)
    m = mask.astype(np.float64)
    positive = g * m
    negative = (1.0 - g) * m
    pos_cnt = positive.sum()
    neg_cnt = min(negative.sum(), np.floor(pos_cnt * NEGATIVE_RATIO))
    loss = ((g - 1.0) * np.log(1.0 - p + LOG_EPS) / np.exp(1.0 - p)
            - g * np.log(p + LOG_EPS) / np.exp(p))
    pos_loss = (loss * positive).sum()
    flat_neg = (loss * negative).ravel()
    k = int(np.ceil(neg_cnt - 1e-12)) if neg_cnt > 0 else 0
    if k >= flat_neg.size:
        neg_sum = flat_neg.sum()
    elif k > 0:
        neg_sum = np.partition(flat_neg, flat_neg.size - k)[flat_neg.size - k:].sum()
    else:
        neg_sum = 0.0
    return np.float32((pos_loss + neg_sum) / (pos_cnt + neg_cnt + EPS))


def kernel(pred, gt, mask):
    pred = np.asarray(pred)
    gt = np.asarray(gt)
    mask = np.asarray(mask)
    if not (mask == 1.0).all() or not ((gt == 0.0) | (gt == 1.0)).all():
        return _fallback(pred, gt, mask)

    pr = np.ascontiguousarray(pred, dtype=np.float32).reshape(N_CORES, P, FREE)
    gr = np.ascontiguousarray(gt, dtype=np.float32).reshape(N_CORES, P, FREE)
    SA, SAg, SBg, GS, _ = _device_sums(pr, gr)

    pos_cnt = GS
    neg_raw = float(TOTAL) - GS
    neg_count = min(neg_raw, float(np.floor(np.float32(pos_cnt) * np.float32(NEGATIVE_RATIO))))
    if neg_raw > neg_count + 0.5:
        # top-k actually bites; take the exact path
        return _fallback(pred, gt, mask)

    pos_loss = -SBg
    neg_sum = SAg - SA
    return np.float32((pos_loss + neg_sum) / (pos_cnt + neg_count + EPS))


# revision 14
# speedup vs baseline: 1.0541x; 1.0541x over previous
"""BalanceCrossEntropyLoss on 8 Trainium2 NeuronCores.

Problem shapes (hardcoded): pred (16,1,1024,1024) f32, gt (16,1,1024,1024) f32,
mask (16,1024,1024) f32. Output: scalar f32.

Strategy
--------
Data-parallel over the flattened 16M elements: each of the 8 cores gets a
contiguous [128, 16384] f32 shard of pred and gt.

For binary gt and all-ones mask (the spec's fill types), the reference's
hard-negative top-k selects *all* negatives whenever
#neg <= floor(3 * #pos) (true with overwhelming probability for p=0.5
Bernoulli gt), because every negative-pixel loss is > 0 and all other entries
of the flattened negative-loss map are exactly 0.  So:

    negative_sum  = sum(negative_loss)      (no sort/top-k needed)
    balance_loss  = (sum(pos_loss) + negative_sum) / (#pos + #neg + eps)

With g in {0,1}, m == 1:
    pos_loss  = g * (-ln(p+eps) * e^-p)         = -B*g,  B = ln(p+eps)*e^-p
    neg_loss  = (1-g) * (-ln(1-p+eps) * e^(p-1)) = -(A - A*g), A = ln(1-p+eps)*e^(p-1)

Per core the kernel computes four scalars-per-partition streams:
    sum(A), sum(A*g), sum(B*g)   via VectorE tensor_tensor_reduce (fused
                                 product + free-dim reduction)
    sum(g)                       via TensorE ones^T @ g accumulated in PSUM
with A,B built from 4 ScalarE activations (Ln, Ln, Exp, Exp w/ scale+bias).

Host preconditions (mask all-ones, gt binary, #neg <= floor(3*#pos)) are
checked; any violation falls back to an exact numpy implementation of the
reference (including the true top-k).
"""

import sys

sys.path.insert(0, "/opt/trn_rl_repo")

import numpy as np

N_CORES = 8
P = 128
FREE = 16384          # per-core free dim: 16M / 8 cores / 128 partitions
F = 4096              # tile free dim
NT = FREE // F
MM_N = 512            # matmul free-dim chunk (one PSUM bank)
TOTAL = 16 * 1024 * 1024
LOG_EPS = 1e-37
NEGATIVE_RATIO = 3.0
EPS = 1e-6

_NC_CACHE = {}


def _patch_act_tables():
    """Restrict Ln/Exp to the combined 'natural_log_exp_and_others' table so
    the act-table-load pass emits one hoisted load instead of per-tile
    switches.  Only affects the copy handed to insert_act_table_loads; the
    table ids still index the compiler's own act_info.json."""
    import concourse.bacc as bacc_mod
    import concourse.mybir as mybir
    from concourse.hw_specs import get_activation_tables as _real

    if getattr(bacc_mod, "_act_tables_patched", False):
        return

    AF = mybir.ActivationFunctionType

    def _combined(arch):
        out = {}
        for name, funcs in _real(arch).items():
            if name == "natural_log_exp_and_others":
                out[name] = set(funcs)
            else:
                out[name] = set(funcs) - {AF.Ln, AF.Exp}
        return out

    bacc_mod.get_activation_tables = _combined
    bacc_mod._act_tables_patched = True


def _build_nc(free=FREE, tile_f=F, debug=False, bf16=True):
    import concourse.bacc as bacc
    import concourse.mybir as mybir
    from concourse.tile import TileContext

    f32 = mybir.dt.float32
    AF = mybir.ActivationFunctionType
    ALU = mybir.AluOpType
    NT = free // tile_f
    F = tile_f
    MM_N = min(512, F)
    work_dt = mybir.dt.bfloat16 if bf16 else f32
    g_dt = mybir.dt.bfloat16 if bf16 else f32

    _patch_act_tables()
    nc = bacc.Bacc(None, target_bir_lowering=False, debug=debug)
    pred = nc.declare_dram_parameter("pred", [P, free], f32, isOutput=False)
    gt = nc.declare_dram_parameter("gt", [P, free], f32, isOutput=False)
    # acc columns: [0:NT]=sum(A), [NT:2NT]=sum(A*g), [2NT:3NT]=sum(B*g),
    # [3NT:4NT]=sum(B) (unused by host, accum_out is mandatory)
    acc_out = nc.declare_dram_parameter("acc", [P, 4 * NT], f32, isOutput=True)
    gsum_out = nc.declare_dram_parameter("gsum", [P, 1], f32, isOutput=True)

    with TileContext(nc) as tc:
        with (
            tc.tile_pool(name="const", bufs=1) as cpool,
            tc.tile_pool(name="io", bufs=3) as io,
            tc.tile_pool(name="work", bufs=2) as work,
            tc.tile_pool(name="psum", bufs=1, space="PSUM") as pp,
        ):
            ones = cpool.tile([P, P], g_dt)
            nc.vector.memset(ones[:], 1.0)
            acc = cpool.tile([P, 4 * NT], f32)
            ps = pp.tile([P, MM_N], f32)

            def const_ap(val, tag):
                t = cpool.tile([P, 1], f32, tag=tag)
                nc.vector.memset(t[:], val)
                return t

            c_eps = const_ap(LOG_EPS, "c_eps")
            c_neg1 = const_ap(-1.0, "c_neg1")

            for i in range(NT):
                sl = slice(i * F, (i + 1) * F)  # noqa: F841 (slice per tile)
                p_t = io.tile([P, F], f32, tag="p")
                g_t = io.tile([P, F], g_dt, tag="g")
                nc.sync.dma_start(out=p_t[:], in_=pred[:, sl])
                # SWDGE casts f32->bf16 in flight (HWDGE can't cast)
                g_dma = nc.gpsimd if g_dt != f32 else nc.sync
                g_dma.dma_start(out=g_t[:], in_=gt[:, sl])

                ln1p = work.tile([P, F], work_dt, tag="ln1p")
                lp = work.tile([P, F], work_dt, tag="lp")
                ep1 = work.tile([P, F], work_dt, tag="ep1")
                enp = work.tile([P, F], work_dt, tag="enp")
                A = work.tile([P, F], work_dt, tag="A")
                B = work.tile([P, F], work_dt, tag="B")

                # ln(1-p) (the +1e-37 is absorbed by f32 rounding, matching
                # the reference's own f32 arithmetic), ln(p+1e-37), e^(p-1), e^-p
                nc.scalar.activation(ln1p[:], p_t[:], AF.Ln, bias=1.0, scale=c_neg1[:])
                nc.scalar.activation(lp[:], p_t[:], AF.Ln, bias=c_eps[:], scale=1.0)
                nc.scalar.activation(ep1[:], p_t[:], AF.Exp, bias=c_neg1[:], scale=1.0)
                nc.scalar.activation(enp[:], p_t[:], AF.Exp, bias=0.0, scale=c_neg1[:])

                # A = ln1p*ep1 (+ running sum); A*g; B = lp*enp; B*g
                # (affine_mul_reduce: out = (in0*scale+bias)*in1, accum = sum(out))
                nc.vector.affine_mul_reduce(
                    out=A[:], in0=ln1p[:], in1=ep1[:], scale=1.0, bias=0.0,
                    accum_out=acc[:, 0 * NT + i : 0 * NT + i + 1])
                nc.vector.affine_mul_reduce(
                    out=ep1[:], in0=A[:], in1=g_t[:], scale=1.0, bias=0.0,
                    accum_out=acc[:, 1 * NT + i : 1 * NT + i + 1])
                nc.vector.affine_mul_reduce(
                    out=B[:], in0=lp[:], in1=enp[:], scale=1.0, bias=0.0,
                    accum_out=acc[:, 3 * NT + i : 3 * NT + i + 1])
                nc.vector.affine_mul_reduce(
                    out=enp[:], in0=B[:], in1=g_t[:], scale=1.0, bias=0.0,
                    accum_out=acc[:, 2 * NT + i : 2 * NT + i + 1])

                # sum(g): ones^T @ g chunks accumulated into one PSUM bank
                for c in range(F // MM_N):
                    nc.tensor.matmul(
                        ps[:, :], ones[:, :], g_t[:, c * MM_N : (c + 1) * MM_N],
                        start=(i == 0 and c == 0),
                        stop=(i == NT - 1 and c == F // MM_N - 1))

            # every psum row holds the same per-column counts; reduce free dim
            gs_t = cpool.tile([P, 1], f32)
            nc.vector.reduce_sum(out=gs_t[:], in_=ps[:, :],
                                 axis=mybir.AxisListType.X)
            nc.sync.dma_start(out=acc_out[:, :], in_=acc[:])
            nc.sync.dma_start(out=gsum_out[:, :], in_=gs_t[:])

    nc.finalize()
    return nc


def _get_nc():
    if "nc" not in _NC_CACHE:
        _NC_CACHE["nc"] = _build_nc()
    return _NC_CACHE["nc"]


def _device_sums(pred32, gt32, trace=False, tmpdir=None):
    """pred32/gt32: (8,128,16384) f32. Returns (SA, SAg, SBg, GS, results)."""
    from concourse.bass_utils import run_bass_kernel_spmd

    nc = _get_nc()
    in_maps = [{"pred": pred32[c], "gt": gt32[c]} for c in range(N_CORES)]
    res = run_bass_kernel_spmd(
        nc, in_maps, core_ids=list(range(N_CORES)), trace=trace, tmpdir=tmpdir)
    SA = SAg = SBg = GS = 0.0
    for c in range(N_CORES):
        a = res.results[c]["acc"].astype(np.float64)
        SA += a[:, 0 * NT : 1 * NT].sum()
        SAg += a[:, 1 * NT : 2 * NT].sum()
        SBg += a[:, 2 * NT : 3 * NT].sum()
        GS += float(res.results[c]["gsum"][0, 0])
    return SA, SAg, SBg, GS, res


def _fallback(pred, gt, mask):
    """Exact numpy mirror of the reference (handles arbitrary inputs)."""
    p = pred[:, 0].astype(np.float64)
    g = gt[:, 0].astype(np.float64)
    m = mask.astype(np.float64)
    positive = g * m
    negative = (1.0 - g) * m
    pos_cnt = positive.sum()
    neg_cnt = min(negative.sum(), np.floor(pos_cnt * NEGATIVE_RATIO))
    loss = ((g - 1.0) * np.log(1.0 - p + LOG_EPS) / np.exp(1.0 - p)
            - g * np.log(p + LOG_EPS) / np.exp(p))
    pos_loss = (loss * positive).sum()
    flat_neg = (loss * negative).ravel()
    k = int(np.ceil(neg_cnt - 1e-12)) if neg_cnt > 0 else 0
    if k >= flat_neg.size:
        neg_sum = flat_neg.sum()
    elif k > 0:
        neg_sum = np.partition(flat_neg, flat_neg.size - k)[flat_neg.size - k:].sum()
    else:
        neg_sum = 0.0
    return np.float32((pos_loss + neg_sum) / (pos_cnt + neg_cnt + EPS))


def kernel(pred, gt, mask):
    pred = np.asarray(pred)
    gt = np.asarray(gt)
    mask = np.asarray(mask)
    if not (mask == 1.0).all() or not ((gt == 0.0) | (gt == 1.0)).all():
        return _fallback(pred, gt, mask)

    pr = np.ascontiguousarray(pred, dtype=np.float32).reshape(N_CORES, P, FREE)
    gr = np.ascontiguousarray(gt, dtype=np.float32).reshape(N_CORES, P, FREE)
    SA, SAg, SBg, GS, _ = _device_sums(pr, gr)

    pos_cnt = GS
    neg_raw = float(TOTAL) - GS
    neg_count = min(neg_raw, float(np.floor(np.float32(pos_cnt) * np.float32(NEGATIVE_RATIO))))
    if neg_raw > neg_count + 0.5:
        # top-k actually bites; take the exact path
        return _fallback(pred, gt, mask)

    pos_loss = -SBg
    neg_sum = SAg - SA
    return np.float32((pos_loss + neg_sum) / (pos_cnt + neg_count + EPS))
